# revision 1
# baseline (speedup 1.0000x reference)
"""Trainium2 Bass kernel for nn_ChannelMixing (RWKV-style channel mixing).

Math: the reference's FFT decay-conv is the first-order IIR
    h[t] = mix*h[t-1] + x[t],  h[-1] = last_x/(1-mix)
and x_mix = (1-mix)*h, so with weights pre-scaled by (1-mix):
    k = h_k @ (Wk*(1-mix_k)).T,  r = h_r @ (Wr*(1-mix_r)).T
    out = sigmoid(r) * (relu(k)^2 @ Wv.T)

Sharding: time dimension L=4096 split over 8 cores (512 rows each) with a
64-step halo to warm up the scan state (decay <= sigmoid(1) ~ 0.731, so
carry across 64 steps < 3e-9 — below fp32 noise). Core 0 gets the exact
initial state via a per-core init column; no collectives.

Layout: everything [channel(P), time(F)]. The scan runs on the vector
engine (tensor_tensor_scan), the three 2048x2048 matmuls on the PE in
fp32r, activations on ACT, gating on DVE.
"""
import numpy as np
from contextlib import ExitStack

import concourse.bass as bass
from concourse import bacc
import concourse.tile as tile
import concourse.mybir as mybir
from concourse.bass_utils import run_bass_kernel_spmd

LEN, DIM = 4096, 2048
NCORES = 8
P = 128
HALO = 64

f32 = mybir.dt.float32
f32r = mybir.dt.float32r
Alu = mybir.AluOpType
Act = mybir.ActivationFunctionType

_cache = {}


def _build(dim, tloc, halo):
    """Build + compile the per-core SPMD program."""
    nt = dim // P          # channel tiles
    ts = tloc + halo       # time slab incl. halo
    ng = max(1, (dim // P) // 4)   # output m-groups of 4 m-tiles
    NF = 512 if tloc >= 512 else tloc   # matmul moving size (time)
    assert tloc % NF == 0
    nf = tloc // NF        # time blocks per matmul (1 at full size)

    nc = bacc.Bacc(trn_type="TRN2", debug=False)

    xs_d = nc.dram_tensor("xs", [dim, ts], f32, kind="ExternalInput").ap()
    dec_d = nc.dram_tensor("dec", [P, 2 * nt], f32, kind="ExternalInput").ap()  # SBUF image
    ini_d = nc.dram_tensor("ini", [P, 2 * nt], f32, kind="ExternalInput").ap()
    wk_d = nc.dram_tensor("wk", [dim, dim], f32r, kind="ExternalInput").ap()  # [d, i] pre-scaled
    wr_d = nc.dram_tensor("wr", [dim, dim], f32r, kind="ExternalInput").ap()
    wv_d = nc.dram_tensor("wv", [dim, dim], f32r, kind="ExternalInput").ap()  # [i, o]
    out_d = nc.dram_tensor("out", [dim, tloc], f32, kind="ExternalOutput").ap()

    with tile.TileContext(nc) as tc, ExitStack() as ctx:
        const = ctx.enter_context(tc.tile_pool(name="const", bufs=1))
        xs_pool = ctx.enter_context(tc.tile_pool(name="xs", bufs=6))
        h_pool = ctx.enter_context(tc.tile_pool(name="h", bufs=1))
        w_pool = ctx.enter_context(tc.tile_pool(name="w", bufs=12))
        ev_pool = ctx.enter_context(tc.tile_pool(name="ev", bufs=1))
        sc_pool = ctx.enter_context(tc.tile_pool(name="sc", bufs=3))
        o_pool = ctx.enter_context(tc.tile_pool(name="o", bufs=3))
        ps_pool = ctx.enter_context(tc.tile_pool(name="ps", bufs=2, space="PSUM"))

        # per-channel constants: [P, nt] tiles (col ct = chan tile ct)
        dec_t = const.tile([P, 2 * nt], f32)
        nc.scalar.dma_start(dec_t[:], dec_d)
        ini_t = const.tile([P, 2 * nt], f32)
        nc.scalar.dma_start(ini_t[:], ini_d)

        # ---- stage A: decay scans -> h_k, h_r in [chan, time] ----
        h = {"k": [None] * nt, "r": [None] * nt}
        for pi, p in enumerate(("k", "r")):
            for ct in range(nt):
                xs = xs_pool.tile([P, ts], f32, tag="xs", name=f"xs{p}{ct}")
                nc.scalar.dma_start(xs[:], xs_d[ct * P:(ct + 1) * P, :])
                dcol = dec_t[:, 2 * ct + pi: 2 * ct + pi + 1]
                # single scan over halo+body; core0's initial state is
                # h0*mix^-halo (host-prepped) so it decays to exactly h0
                # across the zero halo columns.
                hs = h_pool.tile([P, ts], f32r, tag=f"h{p}{ct}", name=f"hs{p}{ct}")
                nc.vector.tensor_tensor_scan(
                    hs[:], dcol.broadcast_to([P, ts]), xs[:],
                    ini_t[:, 2 * ct + pi: 2 * ct + pi + 1],
                    op0=Alu.mult, op1=Alu.add)
                h[p][ct] = hs[:, halo:]

        # ---- stage B helper: out[o_tile, t] = sum_kt w[kt,o].T @ rhs[kt] ----
        def big_matmul(w_dram, rhs_tiles, evict_fn, wtag):
            for g in range(ng):
                m4 = min(4, nt - 4 * g)
                psums = [ps_pool.tile([P, NF], f32, tag=f"ps{m}",
                                      name=f"ps_{wtag}_{g}_{m}") for m in range(m4)]
                for tb in range(nf):
                    for kt in range(nt):
                        wt = w_pool.tile([P, m4 * P], f32r, tag="w",
                                         name=f"wt_{wtag}_{g}_{kt}")
                        nc.sync.dma_start(
                            wt[:], w_dram[kt * P:(kt + 1) * P,
                                          g * 4 * P: g * 4 * P + m4 * P])
                        for m in range(m4):
                            nc.tensor.matmul(
                                psums[m][:], wt[:, m * P:(m + 1) * P],
                                rhs_tiles[kt][:, tb * NF:(tb + 1) * NF],
                                start=(kt == 0), stop=(kt == nt - 1))
                    for m in range(m4):
                        evict_fn(g * 4 + m, tb, psums[m])

        # k path: evict = relu then square -> sq tiles (f32r)
        sq = [ev_pool.tile([P, tloc], f32r, tag=f"sq{i}", name=f"sq{i}") for i in range(nt)]

        def evict_k(mi, tb, psum):
            rr = sc_pool.tile([P, NF], f32, tag="rr")
            nc.scalar.activation(rr[:], psum[:], Act.Relu)
            nc.vector.tensor_mul(sq[mi][:, tb * NF:(tb + 1) * NF], rr[:], rr[:])

        # r path: evict = sigmoid -> sig tiles (f32)
        sig = [ev_pool.tile([P, tloc], f32, tag=f"sg{i}", name=f"sg{i}") for i in range(nt)]

        def evict_r(mi, tb, psum):
            nc.scalar.activation(sig[mi][:, tb * NF:(tb + 1) * NF], psum[:], Act.Sigmoid)

        # v path: evict = gate with sigmoid(r) -> DMA out
        def evict_v(mi, tb, psum):
            ot = o_pool.tile([P, NF], f32, tag="ot")
            nc.vector.tensor_mul(ot[:], psum[:], sig[mi][:, tb * NF:(tb + 1) * NF])
            nc.sync.dma_start(out_d[mi * P:(mi + 1) * P, tb * NF:(tb + 1) * NF], ot[:])

        # PE warmup during the scan phase: keeps HAM at K=8/8 so the real
        # matmul stream starts warm. Uses the first weight tile as both
        # operands; results are discarded (psum slot reused with start=True).
        wsz = min(NF, dim)
        wm = min(P, wsz)
        wt0 = w_pool.tile([P, wsz], f32r, tag="w", name="wt_warm")
        nc.sync.dma_start(wt0[:], wk_d[0:P, 0:wsz])
        ps_w = ps_pool.tile([P, wsz], f32, tag="ps0", name="ps_warm")
        for _ in range(28):
            nc.tensor.matmul(ps_w[0:wm, :], wt0[:, 0:wm],
                             wt0[:], start=True, stop=True)

        big_matmul(wk_d, h["k"], evict_k, "wk")
        big_matmul(wr_d, h["r"], evict_r, "wr")
        big_matmul(wv_d, sq, evict_v, "wv")

    nc.compile()
    return nc


def _sigmoid(v):
    return 1.0 / (1.0 + np.exp(-v.astype(np.float64)))


def _prep(x, Wk, Wr, Wv, mix_k, mix_r, lxk, lxr, ncores, halo):
    """Host-side prep: transposes, weight pre-scaling, per-core slabs."""
    dim = x.shape[1]
    tloc = x.shape[0] // ncores
    mk = _sigmoid(mix_k).astype(np.float32)
    mr = _sigmoid(mix_r).astype(np.float32)
    h0k = (lxk / (1.0 - mk)).astype(np.float32)
    h0r = (lxr / (1.0 - mr)).astype(np.float32)
    P = 128
    nt = dim // P
    dec = np.empty((P, 2 * nt), np.float32)   # SBUF image: [p, 2*ct+path]
    dec[:, 0::2] = mk.reshape(nt, P).T
    dec[:, 1::2] = mr.reshape(nt, P).T

    wk = np.ascontiguousarray((Wk * (1.0 - mk)[None, :]).T.astype(np.float32))
    wr = np.ascontiguousarray((Wr * (1.0 - mr)[None, :]).T.astype(np.float32))
    wv = np.ascontiguousarray(Wv.T.astype(np.float32))

    xT = np.ascontiguousarray(x.T.astype(np.float32))       # [dim, L]
    in_maps = []
    for c in range(ncores):
        t0 = c * tloc
        slab = np.empty((dim, halo + tloc), np.float32)
        if c == 0:
            slab[:, :halo] = 0.0
            bk = (h0k.astype(np.float64) * (1.0 / mk.astype(np.float64)) ** halo
                  ).astype(np.float32)
            br = (h0r.astype(np.float64) * (1.0 / mr.astype(np.float64)) ** halo
                  ).astype(np.float32)
            ini = np.empty((P, 2 * nt), np.float32)
            ini[:, 0::2] = bk.reshape(nt, P).T
            ini[:, 1::2] = br.reshape(nt, P).T
        else:
            slab[:, :halo] = xT[:, t0 - halo: t0]
            ini = np.zeros((P, 2 * nt), np.float32)
        slab[:, halo:] = xT[:, t0: t0 + tloc]
        in_maps.append({
            "xs": slab, "dec": dec, "ini": np.ascontiguousarray(ini),
            "wk": wk, "wr": wr, "wv": wv,
        })
    return in_maps


def kernel(x, Wk, Wr, Wv, mix_k, mix_r, last_x_mix_k, last_x_mix_r):
    x = np.asarray(x, np.float32)
    Wk = np.asarray(Wk, np.float32)
    Wr = np.asarray(Wr, np.float32)
    Wv = np.asarray(Wv, np.float32)
    mix_k = np.asarray(mix_k, np.float32)
    mix_r = np.asarray(mix_r, np.float32)
    lxk = np.asarray(last_x_mix_k, np.float32)
    lxr = np.asarray(last_x_mix_r, np.float32)

    L, dim = x.shape
    tloc = L // NCORES
    key = (dim, tloc, HALO)
    if key not in _cache:
        _cache[key] = _build(dim, tloc, HALO)
    nc = _cache[key]

    in_maps = _prep(x, Wk, Wr, Wv, mix_k, mix_r, lxk, lxr, NCORES, HALO)
    # First execution on a cold device occasionally returns
    # NRT_EXEC_UNIT_UNRECOVERABLE; a retry has always succeeded.
    res = None
    for attempt in range(3):
        try:
            res = run_bass_kernel_spmd(nc, in_maps, core_ids=list(range(NCORES)))
            break
        except Exception:
            if attempt == 2:
                raise

    out = np.empty((L, dim), np.float32)
    for c in range(NCORES):
        out[c * tloc:(c + 1) * tloc, :] = res.results[c]["out"].T
    return out



# revision 3
# speedup vs baseline: 1.1715x; 1.1715x over previous
"""Trainium2 Bass kernel for nn_ChannelMixing (RWKV-style channel mixing).

Math: the reference's FFT decay-conv is the first-order IIR
    h[t] = mix*h[t-1] + x[t],  h[-1] = last_x/(1-mix)
and x_mix = (1-mix)*h, so with weights pre-scaled by (1-mix):
    k = h_k @ (Wk*(1-mix_k)).T,  r = h_r @ (Wr*(1-mix_r)).T
    out = sigmoid(r) * (relu(k)^2 @ Wv.T)

Sharding: time dimension L=4096 split over 8 cores (512 rows each) with a
64-step halo to warm up the scan state (decay <= sigmoid(1) ~ 0.731, so
carry across 64 steps < 3e-9 — below the tolerance). Core 0 gets the exact
initial state via a per-core init column; no collectives.

v2 layout: all matmul operands bf16 (halves the 50 MB/core weight DMA that
made v1 DMA-bound; scan state stays fp32 internally, only h is rounded).
The three GEMMs run as 48 16-deep same-bank PSUM accumulation chains,
skewed 2 steps apart across the 8 banks so chain closures (and their
ACT-side evictions) stagger instead of stalling the PE at phase edges.
Weight tiles [128, 1024] bf16 stream through a small pool in exact
consumption order. xs is loaded once and shared by the k and r scans.
"""
import numpy as np
import ml_dtypes
from contextlib import ExitStack

import concourse.bass as bass
from concourse import bacc
import concourse.tile as tile
import concourse.mybir as mybir
from concourse.bass_utils import run_bass_kernel_spmd

LEN, DIM = 4096, 2048
NCORES = 8
P = 128
HALO = 64

f32 = mybir.dt.float32
bf16 = mybir.dt.bfloat16
Alu = mybir.AluOpType
Act = mybir.ActivationFunctionType

_cache = {}


def _build(dim, tloc, halo):
    """Build + compile the per-core SPMD program."""
    nt = dim // P          # 16 channel tiles
    ts = tloc + halo       # time slab incl. halo
    NF = tloc              # 512: matmul moving size == psum bank
    SW = 2                 # skew (in steps) between adjacent psum banks

    nc = bacc.Bacc(trn_type="TRN2", debug=False)

    xs_d = nc.dram_tensor("xs", [dim, ts], f32, kind="ExternalInput").ap()
    dec_d = nc.dram_tensor("dec", [P, 2 * nt], f32, kind="ExternalInput").ap()
    ini_d = nc.dram_tensor("ini", [P, 2 * nt], f32, kind="ExternalInput").ap()
    wk_d = nc.dram_tensor("wk", [dim, dim], bf16, kind="ExternalInput").ap()  # [i, o]
    wr_d = nc.dram_tensor("wr", [dim, dim], bf16, kind="ExternalInput").ap()
    wv_d = nc.dram_tensor("wv", [dim, dim], bf16, kind="ExternalInput").ap()  # [o1, o2]
    out_d = nc.dram_tensor("out", [dim, tloc], f32, kind="ExternalOutput").ap()

    with tile.TileContext(nc) as tc, ExitStack() as ctx:
        const = ctx.enter_context(tc.tile_pool(name="const", bufs=1))
        xs_pool = ctx.enter_context(tc.tile_pool(name="xs", bufs=nt))
        h_pool = ctx.enter_context(tc.tile_pool(name="h", bufs=1))
        w_pool = ctx.enter_context(tc.tile_pool(name="w", bufs=18))
        sq_pool = ctx.enter_context(tc.tile_pool(name="sq", bufs=1))
        sg_pool = ctx.enter_context(tc.tile_pool(name="sg", bufs=1))
        rr_pool = ctx.enter_context(tc.tile_pool(name="rr", bufs=2))
        o_pool = ctx.enter_context(tc.tile_pool(name="o", bufs=3))
        ps_pool = ctx.enter_context(tc.tile_pool(name="ps", bufs=1, space="PSUM"))

        dec_t = const.tile([P, 2 * nt], f32)
        nc.scalar.dma_start(dec_t[:], dec_d)
        ini_t = const.tile([P, 2 * nt], f32)
        nc.scalar.dma_start(ini_t[:], ini_d)

        # ---- weight tile stream (sync queue), exact consumption order ----
        wtiles = {}
        for X, wd in enumerate((wk_d, wr_d, wv_d)):
            for half in range(2):
                for k in range(nt):
                    wt = w_pool.tile([P, 1024], bf16, tag="w",
                                     name=f"w{X}_{half}_{k}")
                    nc.sync.dma_start(
                        wt[:], wd[k * P:(k + 1) * P,
                                  half * 1024:(half + 1) * 1024])
                    wtiles[(X, half, k)] = wt

        # ---- scans (DVE): xs loaded once, k then r path ----
        xs_t = []
        for ct in range(nt):
            xs = xs_pool.tile([P, ts], f32, tag="xs", name=f"xs{ct}")
            nc.scalar.dma_start(xs[:], xs_d[ct * P:(ct + 1) * P, :])
            xs_t.append(xs)
        h = {0: [None] * nt, 1: [None] * nt}   # 0: k-path, 1: r-path
        for pi in (0, 1):
            for ct in range(nt):
                hs = h_pool.tile([P, ts], bf16, tag=f"h{pi}_{ct}",
                                 name=f"h{pi}_{ct}")
                nc.vector.tensor_tensor_scan(
                    hs[:],
                    dec_t[:, 2 * ct + pi: 2 * ct + pi + 1].broadcast_to([P, ts]),
                    xs_t[ct][:],
                    ini_t[:, 2 * ct + pi: 2 * ct + pi + 1],
                    op0=Alu.mult, op1=Alu.add)
                h[pi][ct] = hs

        sq = [None] * nt   # relu(k)^2, bf16 [P, NF], chan-major
        sig = [None] * nt  # sigmoid(r), bf16 [P, NF]

        # ---- PE warmup: ramp p-state on the first weight tile ----
        wt0 = wtiles[(0, 0, 0)]
        ps_w = ps_pool.tile([P, NF], f32, tag="b7", name="ps_warm")
        for _ in range(6):
            nc.tensor.matmul(ps_w[:], wt0[:, 0:P], wt0[:, 0:NF],
                             start=True, stop=True)

        # ---- 48 skewed accumulation chains over 8 psum banks ----
        # chain jj: weight X=jj//16, half=(jj%16)//8, bank=jj%8,
        # o-group g=half*2+(jj%8)//4, m-tile m=jj%4 -> o-tile oidx=g*4+m.
        # Starts at step (jj//8)*16 + bank*SW; step s runs kt = s - start.
        starts = [(jj // 8) * 16 + (jj % 8) * SW for jj in range(48)]
        ps_t = [None] * 48
        nsteps = starts[-1] + nt

        def evict(jj):
            X = jj // 16
            oidx = ((jj % 16) // 8) * 8 + ((jj % 8) // 4) * 4 + jj % 4
            psum = ps_t[jj]
            if X == 0:
                rr = rr_pool.tile([P, NF], f32, tag="rr", name=f"rr{jj}")
                nc.scalar.activation(rr[:], psum[:], Act.Relu)
                sq[oidx] = sq_pool.tile([P, NF], bf16, tag=f"sq{oidx}",
                                        name=f"sq{oidx}")
                nc.scalar.activation(sq[oidx][:], rr[:], Act.Square)
            elif X == 1:
                sig[oidx] = sg_pool.tile([P, NF], bf16, tag=f"sg{oidx}",
                                         name=f"sg{oidx}")
                nc.scalar.activation(sig[oidx][:], psum[:], Act.Sigmoid)
            else:
                ot = o_pool.tile([P, NF], f32, tag="ot", name=f"ot{jj}")
                nc.vector.tensor_mul(ot[:], psum[:], sig[oidx][:])
                nc.scalar.dma_start(out_d[oidx * P:(oidx + 1) * P, :], ot[:])

        for s in range(nsteps):
            for jj in range(48):
                kt = s - starts[jj]
                if kt < 0 or kt >= nt:
                    continue
                X = jj // 16
                half = (jj % 16) // 8
                coff = ((jj % 8) // 4) * 512 + (jj % 4) * P
                wt = wtiles[(X, half, kt)]
                if kt == 0:
                    ps_t[jj] = ps_pool.tile([P, NF], f32, tag=f"b{jj % 8}",
                                            name=f"ps{jj}")
                if X == 0:
                    rhs = h[0][kt][:, halo:halo + NF]
                elif X == 1:
                    rhs = h[1][kt][:, halo:halo + NF]
                else:
                    rhs = sq[kt][:]
                nc.tensor.matmul(ps_t[jj][:], wt[:, coff:coff + P], rhs,
                                 start=(kt == 0), stop=(kt == nt - 1))
                if kt == nt - 1:
                    evict(jj)

    nc.compile()
    return nc


def _sigmoid(v):
    return 1.0 / (1.0 + np.exp(-v.astype(np.float64)))


def _prep(x, Wk, Wr, Wv, mix_k, mix_r, lxk, lxr, ncores, halo):
    """Host-side prep: transposes, weight pre-scaling, per-core slabs."""
    dim = x.shape[1]
    tloc = x.shape[0] // ncores
    mk = _sigmoid(mix_k).astype(np.float32)
    mr = _sigmoid(mix_r).astype(np.float32)
    h0k = (lxk / (1.0 - mk)).astype(np.float32)
    h0r = (lxr / (1.0 - mr)).astype(np.float32)
    nt = dim // P
    dec = np.empty((P, 2 * nt), np.float32)   # SBUF image: [p, 2*ct+path]
    dec[:, 0::2] = mk.reshape(nt, P).T
    dec[:, 1::2] = mr.reshape(nt, P).T

    bf = ml_dtypes.bfloat16
    wk = np.ascontiguousarray((Wk * (1.0 - mk)[None, :]).T).astype(bf)
    wr = np.ascontiguousarray((Wr * (1.0 - mr)[None, :]).T).astype(bf)
    wv = np.ascontiguousarray(Wv.T).astype(bf)

    xT = np.ascontiguousarray(x.T.astype(np.float32))       # [dim, L]
    in_maps = []
    for c in range(ncores):
        t0 = c * tloc
        slab = np.empty((dim, halo + tloc), np.float32)
        if c == 0:
            slab[:, :halo] = 0.0
            bk = (h0k.astype(np.float64) * (1.0 / mk.astype(np.float64)) ** halo
                  ).astype(np.float32)
            br = (h0r.astype(np.float64) * (1.0 / mr.astype(np.float64)) ** halo
                  ).astype(np.float32)
            ini = np.empty((P, 2 * nt), np.float32)
            ini[:, 0::2] = bk.reshape(nt, P).T
            ini[:, 1::2] = br.reshape(nt, P).T
        else:
            slab[:, :halo] = xT[:, t0 - halo: t0]
            ini = np.zeros((P, 2 * nt), np.float32)
        slab[:, halo:] = xT[:, t0: t0 + tloc]
        in_maps.append({
            "xs": slab, "dec": dec, "ini": np.ascontiguousarray(ini),
            "wk": wk, "wr": wr, "wv": wv,
        })
    return in_maps


def kernel(x, Wk, Wr, Wv, mix_k, mix_r, last_x_mix_k, last_x_mix_r):
    x = np.asarray(x, np.float32)
    Wk = np.asarray(Wk, np.float32)
    Wr = np.asarray(Wr, np.float32)
    Wv = np.asarray(Wv, np.float32)
    mix_k = np.asarray(mix_k, np.float32)
    mix_r = np.asarray(mix_r, np.float32)
    lxk = np.asarray(last_x_mix_k, np.float32)
    lxr = np.asarray(last_x_mix_r, np.float32)

    L, dim = x.shape
    tloc = L // NCORES
    key = (dim, tloc, HALO)
    if key not in _cache:
        _cache[key] = _build(dim, tloc, HALO)
    nc = _cache[key]

    in_maps = _prep(x, Wk, Wr, Wv, mix_k, mix_r, lxk, lxr, NCORES, HALO)
    # First execution on a cold device occasionally returns
    # NRT_EXEC_UNIT_UNRECOVERABLE; a retry has always succeeded.
    res = None
    for attempt in range(3):
        try:
            res = run_bass_kernel_spmd(nc, in_maps, core_ids=list(range(NCORES)))
            break
        except Exception:
            if attempt == 2:
                raise

    out = np.empty((L, dim), np.float32)
    for c in range(NCORES):
        out[c * tloc:(c + 1) * tloc, :] = res.results[c]["out"].T
    return out
